# revision 1
# baseline (speedup 1.0000x reference)
"""GCF message passing on 8 trn2 cores — windowed-SpMM redesign.

Per core (dest-shard of 12500 nodes), per layer:
  SpMM: edges sorted by dest, grouped into shared windows (<=128 dests);
        per (window, col-range) up to KMAX chunks of 128 edges. One-hot
        matrices O[slot, dest-d0] (vals folded, fp16) are precomputed on
        host and streamed from DRAM; gathered source rows (fp16, 256B)
        come from HBM dma_gather. PE accumulates Lx^T window = sum over
        chunks G^T @ O in PSUM; DVE casts to fp16 ftout.
  Dense: y^T = Wlin@(Lx+F)^T + Wint@(Lx*F)^T per 512-block, fused
        bias+leaky-relu on ACT, row norm via ones-matmul + reciprocal +
        broadcast-matmul, all fp16 in SBUF / f32 in PSUM.
  Share: PE-transpose shard -> fshard fp16, AllGather -> ags[l].
Final: per concat level gather u/i rows (fp16), multiply + reduce, acc.
"""

import os

import numpy as np

import concourse.bacc as bacc
import concourse.mybir as mybir
import concourse.tile as tile
from concourse.bass_utils import run_bass_kernel_spmd
from concourse.masks import make_identity

NUM_USERS = 30000
NUM_ITEMS = 70000
N = 100000
D = 128
NL = 3
BATCH = 16384
NCORE = 8
SHARD = N // NCORE
RS = 25000
NR = 4
WCAP = 128
KMAX = 3
GM = 64                # target chunks per gather group
BSH = BATCH // NCORE   # 2048
EPS = 1e-12
SLOPE = 0.01

f32 = mybir.dt.float32
f16 = mybir.dt.float16
i16 = mybir.dt.int16

NQUEUES = int(os.environ.get("KQ", "4"))

_cache = {}


# ---------------------------------------------------------------- host side
def _build_structure(rows, cols):
    core = rows // SHARD
    dloc = rows - core * SHARD
    rng = cols // RS

    counts = np.zeros((NCORE, SHARD, NR), np.int32)
    np.add.at(counts, (core, dloc, rng), 1)

    windows = []
    cum = np.zeros((NCORE, NR), np.int64)
    d0 = 0
    for d in range(SHARD):
        c = counts[:, d, :]
        if (cum + c > 128 * KMAX).any() or d - d0 >= WCAP:
            windows.append((d0, d - d0))
            d0 = d
            cum = c.astype(np.int64).copy()
        else:
            cum += c
    windows.append((d0, SHARD - d0))
    nwin = len(windows)

    wsum = np.zeros((nwin, NR, NCORE), np.int64)
    for i, (a, w) in enumerate(windows):
        wsum[i] = counts[:, a:a + w, :].sum(axis=1).T
    kwr = np.ceil(wsum.max(axis=2) / 128).astype(np.int64)
    empty = kwr.sum(axis=1) == 0
    kwr[empty, 0] = 1

    chunks = []  # (win, r, j, is_first, is_last)
    for i in range(nwin):
        ks = kwr[i]
        tot = int(ks.sum())
        seen = 0
        for r in range(NR):
            for j in range(int(ks[r])):
                chunks.append((i, r, j, seen == 0, seen == tot - 1))
                seen += 1
    NCH = len(chunks)

    groups = []
    gstart, cnt = 0, 0
    for i in range(nwin):
        c = int(kwr[i].sum())
        if cnt + c > GM and cnt > 0:
            groups.append((gstart, i))
            gstart, cnt = i, 0
        cnt += c
    groups.append((gstart, nwin))

    ci_of = {}
    for ci, (i, r, j, _, _) in enumerate(chunks):
        ci_of[(i, r, j)] = ci
    chunk_group = np.zeros(NCH, np.int64)
    chunk_slot = np.zeros(NCH, np.int64)
    group_info = []
    for gi, (wa, wb) in enumerate(groups):
        col = 0
        rlist = []
        for r in range(NR):
            base, n_r = col, 0
            for i in range(wa, wb):
                for j in range(int(kwr[i, r])):
                    ci = ci_of[(i, r, j)]
                    chunk_group[ci] = gi
                    chunk_slot[ci] = col
                    col += 1
                    n_r += 1
            if n_r:
                rlist.append((r, base, n_r))
        group_info.append((col, tuple(rlist), (wa, wb)))

    o_off = np.zeros(NCH + 1, np.int64)
    for ci, (i, r, j, _, _) in enumerate(chunks):
        o_off[ci + 1] = o_off[ci] + windows[i][1]
    return dict(windows=tuple(windows), kwr=kwr, chunks=tuple(chunks),
                groups=tuple(groups), group_info=tuple(group_info),
                chunk_group=chunk_group, chunk_slot=chunk_slot,
                o_off=o_off, OW=int(o_off[-1]), NCH=NCH)


def _pack_edges(rows, cols, vals, st):
    core = rows // SHARD
    dloc = rows - core * SHARD
    rng = cols // RS
    windows, kwr = st["windows"], st["kwr"]
    nwin = len(windows)
    NCH = st["NCH"]

    win_of = np.zeros(SHARD, np.int64)
    d0s = np.zeros(nwin, np.int64)
    for i, (a, w) in enumerate(windows):
        win_of[a:a + w] = i
        d0s[i] = a
    win_e = win_of[dloc]

    first_chunk = np.zeros((nwin, NR), np.int64)
    acc = 0
    for i in range(nwin):
        for r in range(NR):
            first_chunk[i, r] = acc
            acc += int(kwr[i, r])

    key = (core * nwin + win_e) * NR + rng
    order = np.argsort(key, kind="stable")
    key_s = key[order]
    uniq, inv, cnt = np.unique(key_s, return_inverse=True, return_counts=True)
    starts = np.zeros(len(uniq), np.int64)
    np.cumsum(cnt[:-1], out=starts[1:])
    pos = np.arange(len(key_s)) - starts[inv]

    core_s = core[order]
    win_s = win_e[order]
    rng_s = rng[order]
    ci_e = first_chunk[win_s, rng_s] + pos // 128
    p_e = pos % 128

    gcols = np.array([g[0] for g in st["group_info"]], np.int64)
    gcol_base = np.zeros(len(gcols) + 1, np.int64)
    np.cumsum(gcols, out=gcol_base[1:])
    gslot = gcol_base[st["chunk_group"][ci_e]] + st["chunk_slot"][ci_e]

    eidx = np.zeros((NCORE, 16, NCH * 8), np.int16)
    col_local = (cols[order] - rng_s * RS).astype(np.int16)
    eidx[core_s, p_e % 16, gslot * 8 + p_e // 16] = col_local

    o_off = st["o_off"]
    O = np.zeros((NCORE, 128, st["OW"]), np.float16)
    j_e = dloc[order] - d0s[win_s]
    O[core_s, p_e, o_off[ci_e] + j_e] = vals[order].astype(np.float16)
    return eidx, O


def _pack_final(userIdx, itemIdx):
    irow = itemIdx + NUM_USERS
    ir = irow // RS
    nfb_counts = np.zeros((NCORE, NR), np.int64)
    perms = []
    for c in range(NCORE):
        sl = slice(c * BSH, (c + 1) * BSH)
        o = np.argsort(ir[sl], kind="stable")
        perms.append(o)
        nfb_counts[c] = np.bincount(ir[sl][o], minlength=NR)
    bucket_chunks = np.ceil(nfb_counts.max(axis=0) / 128).astype(np.int64)
    fin_bounds = []
    c0 = 0
    for r in range(NR):
        n = int(bucket_chunks[r])
        if n == 0:
            continue
        fin_bounds.append((r, c0, n))
        c0 += n
    NFB = c0

    uidx_arr = np.zeros((NCORE, 16, NFB * 8), np.int16)
    iidx_arr = np.zeros((NCORE, 16, NFB * 8), np.int16)
    inv_perm = np.full((NCORE, NFB * 128), -1, np.int64)
    for c in range(NCORE):
        sl = slice(c * BSH, (c + 1) * BSH)
        o = perms[c]
        u_s = userIdx[sl][o]
        i_s = irow[sl][o]
        r_s = ir[sl][o]
        jpos = np.zeros(BSH, np.int64)
        for (r, b0, nchk) in fin_bounds:
            m = r_s == r
            jpos[m] = b0 * 128 + np.arange(int(m.sum()))
        uidx_arr[c, jpos % 16, (jpos // 128) * 8 + (jpos % 128) // 16] = \
            u_s.astype(np.int16)
        iidx_arr[c, jpos % 16, (jpos // 128) * 8 + (jpos % 128) // 16] = \
            (i_s - r_s * RS).astype(np.int16)
        inv_perm[c, jpos] = np.arange(c * BSH, (c + 1) * BSH)[o]
    return NFB, tuple(fin_bounds), uidx_arr, iidx_arr, inv_perm


def _pack_inputs(userIdx, itemIdx, rows, cols, vals, uEmbd, iEmbd,
                 Wlin, blin, Wint, bint):
    rows = np.asarray(rows, dtype=np.int64)
    cols = np.asarray(cols, dtype=np.int64)
    vals = np.asarray(vals, dtype=np.float32)
    userIdx = np.asarray(userIdx, dtype=np.int64)
    itemIdx = np.asarray(itemIdx, dtype=np.int64)

    feat0 = np.concatenate([np.asarray(uEmbd, np.float32),
                            np.asarray(iEmbd, np.float32)], axis=0)
    feat16 = np.ascontiguousarray(feat0.astype(np.float16))

    st = _build_structure(rows, cols)
    eidx, O = _pack_edges(rows, cols, vals, st)
    NFB, fin_bounds, uidx_arr, iidx_arr, inv_perm = _pack_final(
        userIdx, itemIdx)

    wlin_h = np.ascontiguousarray(
        np.asarray(Wlin, np.float32).transpose(1, 0, 2).reshape(D, NL * D)
    ).astype(np.float16)
    wint_h = np.ascontiguousarray(
        np.asarray(Wint, np.float32).transpose(1, 0, 2).reshape(D, NL * D)
    ).astype(np.float16)
    biasc = np.ascontiguousarray(
        (np.asarray(blin, np.float32) + np.asarray(bint, np.float32)).T)

    in_maps = []
    for c in range(NCORE):
        f0t = np.ascontiguousarray(
            feat16[c * SHARD:(c + 1) * SHARD].T)
        in_maps.append({
            "feat16": feat16,
            "f0t": f0t,
            "eidx": np.ascontiguousarray(np.tile(eidx[c], (8, 1))),
            "odram": np.ascontiguousarray(O[c]),
            "wlin": wlin_h,
            "wint": wint_h,
            "biasc": biasc,
            "uidx": np.ascontiguousarray(np.tile(uidx_arr[c], (8, 1))),
            "iidx": np.ascontiguousarray(np.tile(iidx_arr[c], (8, 1))),
        })

    meta = (st, NFB, fin_bounds)
    return meta, in_maps, inv_perm


# ---------------------------------------------------------------- kernel
def _build(meta):
    st, NFB, fin_bounds = meta
    windows = st["windows"]
    chunks = st["chunks"]
    group_info = st["group_info"]
    o_off = st["o_off"]
    kwr = st["kwr"]
    NCH = st["NCH"]
    OW = st["OW"]
    nwin = len(windows)

    gcols = [g[0] for g in group_info]
    maxgc = max(gcols)
    gcol_base = np.zeros(len(gcols) + 1, np.int64)
    np.cumsum(gcols, out=gcol_base[1:])

    # per-group O slice bounds (window-major chunk packing is contiguous)
    first_chunk = np.zeros((nwin, NR), np.int64)
    acc = 0
    for i in range(nwin):
        for r in range(NR):
            first_chunk[i, r] = acc
            acc += int(kwr[i, r])
    grp_obase = []
    grp_osize = []
    for (wa, wb) in st["groups"]:
        c_first = int(first_chunk[wa, 0])
        c_last = int(first_chunk[wb, 0]) if wb < nwin else NCH
        grp_obase.append(int(o_off[c_first]))
        grp_osize.append(int(o_off[c_last] - o_off[c_first]))
    maxgo = max(grp_osize)

    nc = bacc.Bacc(num_devices=NCORE, num_swdge_queues=max(NQUEUES, 1))

    feat16 = nc.dram_tensor("feat16", [N, D], f16, kind="ExternalInput")
    f0t = nc.dram_tensor("f0t", [D, SHARD], f16, kind="ExternalInput")
    eidx = nc.dram_tensor("eidx", [128, NCH * 8], i16, kind="ExternalInput")
    odram = nc.dram_tensor("odram", [128, OW], f16, kind="ExternalInput")
    wlin = nc.dram_tensor("wlin", [D, NL * D], f16, kind="ExternalInput")
    wint = nc.dram_tensor("wint", [D, NL * D], f16, kind="ExternalInput")
    biasc = nc.dram_tensor("biasc", [D, NL], f32, kind="ExternalInput")
    uidx = nc.dram_tensor("uidx", [128, NFB * 8], i16, kind="ExternalInput")
    iidx = nc.dram_tensor("iidx", [128, NFB * 8], i16, kind="ExternalInput")
    score = nc.dram_tensor("score", [128, NFB], f32, kind="ExternalOutput")

    add = mybir.AluOpType.add
    mult = mybir.AluOpType.mult
    maxop = mybir.AluOpType.max
    AF = mybir.ActivationFunctionType

    with tile.TileContext(nc) as tc:
        with (
            tc.tile_pool(name="const", bufs=1) as cp,
            tc.tile_pool(name="ft", bufs=1) as ftp,
            tc.tile_pool(name="g", bufs=3) as gp,
            tc.tile_pool(name="ob", bufs=2) as op_,
            tc.tile_pool(name="sb", bufs=3) as sbp,
            tc.tile_pool(name="fin", bufs=2) as fp_,
            tc.tile_pool(name="pw", bufs=4, space="PSUM") as pwp,
            tc.tile_pool(name="py", bufs=2, space="PSUM") as pyp,
            tc.tile_pool(name="pn", bufs=1, space="PSUM") as pnp,
            tc.tile_pool(name="pbc", bufs=1, space="PSUM") as pbc,
            tc.tile_pool(name="dram", bufs=1, space="DRAM") as dp,
        ):
            # ---------- constants ----------
            ident = cp.tile([128, 128], f16)
            make_identity(nc, ident[:])
            ones_c = cp.tile([128, 1], f16)
            nc.vector.memset(ones_c[:], 1.0)
            onesrow = cp.tile([1, 128], f16)
            nc.vector.memset(onesrow[:], 1.0)

            wlin_sb = cp.tile([128, NL * 128], f16)
            nc.sync.dma_start(out=wlin_sb[:], in_=wlin[:])
            wint_sb = cp.tile([128, NL * 128], f16)
            nc.sync.dma_start(out=wint_sb[:], in_=wint[:])
            bias_sb = cp.tile([128, NL], f32)
            nc.sync.dma_start(out=bias_sb[:], in_=biasc[:])

            eidx_sb = cp.tile([128, NCH * 8], i16)
            nc.sync.dma_start(out=eidx_sb[:], in_=eidx[:])
            uidx_sb = cp.tile([128, NFB * 8], i16)
            nc.sync.dma_start(out=uidx_sb[:], in_=uidx[:])
            iidx_sb = cp.tile([128, NFB * 8], i16)
            nc.sync.dma_start(out=iidx_sb[:], in_=iidx[:])

            FTP = 12544  # SHARD padded to x128 for PE transposes
            fta = ftp.tile([128, FTP], f16, tag="fta")
            ftb = ftp.tile([128, FTP], f16, tag="ftb")
            nc.vector.memset(fta[:, SHARD:], 0.0)
            nc.vector.memset(ftb[:, SHARD:], 0.0)
            nc.sync.dma_start(out=fta[:, :SHARD], in_=f0t[:])

            fshard = dp.tile([SHARD, D], f16)
            ags = [dp.tile([N, D], f16, name=f"ag{i}", tag=f"ag{i}",
                           addr_space="Shared") for i in range(NL)]

            for l in range(NL):
                ftin = fta if l % 2 == 0 else ftb
                ftout = ftb if l % 2 == 0 else fta
                src = feat16 if l == 0 else ags[l - 1]

                # ---------- SpMM ----------
                blocks_done = 0
                for gi, (ncols, rlist, (wa, wb)) in enumerate(group_info):
                    gt = gp.tile([128, maxgc * 128], f16, tag="gt",
                                 name=f"gt{l}_{gi}")
                    for (r, cbase, n_r) in rlist:
                        q = (r % NQUEUES) if NQUEUES > 1 else 0
                        nc.gpsimd.dma_gather(
                            gt[:, cbase * 128:(cbase + n_r) * 128].rearrange(
                                "p (c d) -> p c d", d=128),
                            src[r * RS:, :],
                            eidx_sb[:, (gcol_base[gi] + cbase) * 8:
                                    (gcol_base[gi] + cbase + n_r) * 8],
                            n_r * 128, n_r * 128, 128,
                            single_packet=False, queue_num=q,
                        )
                    ob = op_.tile([128, maxgo], f16, tag="ob",
                                  name=f"ob{l}_{gi}")
                    nc.sync.dma_start(
                        out=ob[:, :grp_osize[gi]],
                        in_=odram[:, grp_obase[gi]:
                                  grp_obase[gi] + grp_osize[gi]])

                    for i in range(wa, wb):
                        d0, w = windows[i]
                        ps = pwp.tile([128, w], f32, tag="pw",
                                      name=f"pw{l}_{i}")
                        for r in range(NR):
                            for j in range(int(kwr[i, r])):
                                ci = int(first_chunk[i, r]) + j
                                _, _, _, isf, isl = chunks[ci]
                                sc = int(st["chunk_slot"][ci])
                                oloc = int(o_off[ci]) - grp_obase[gi]
                                nc.tensor.matmul(
                                    out=ps[:],
                                    lhsT=gt[:, sc * 128:(sc + 1) * 128],
                                    rhs=ob[:, oloc:oloc + w],
                                    start=isf, stop=isl,
                                )
                        if i % 2 == 0:
                            nc.vector.tensor_copy(ftout[:, d0:d0 + w], ps[:])
                        else:
                            nc.scalar.activation(
                                out=ftout[:, d0:d0 + w], in_=ps[:],
                                func=AF.Copy)

                    # ---------- dense blocks now fully covered ----------
                    if gi + 1 < len(group_info):
                        nw = group_info[gi + 1][2][0]
                        dest_end = windows[nw][0]
                    else:
                        dest_end = SHARD
                    while (blocks_done + 512 <= dest_end
                           or (gi + 1 == len(group_info)
                               and blocks_done < SHARD)):
                        b0 = blocks_done
                        blocks_done += 512
                        w = min(512, SHARD - b0)
                        lxs = ftout[:, b0:b0 + w]
                        fin_ = ftin[:, b0:b0 + w]
                        pre1 = sbp.tile([128, 512], f16, tag="pre1")
                        nc.vector.tensor_tensor(out=pre1[:, :w], in0=lxs,
                                                in1=fin_, op=add)
                        pre2 = sbp.tile([128, 512], f16, tag="pre2")
                        nc.vector.tensor_tensor(out=pre2[:, :w], in0=lxs,
                                                in1=fin_, op=mult)
                        y = pyp.tile([128, 512], f32, tag="y")
                        nc.tensor.matmul(out=y[:, :w],
                                         lhsT=wlin_sb[:, l * 128:(l + 1) * 128],
                                         rhs=pre1[:, :w], start=True, stop=False)
                        nc.tensor.matmul(out=y[:, :w],
                                         lhsT=wint_sb[:, l * 128:(l + 1) * 128],
                                         rhs=pre2[:, :w], start=False, stop=True)
                        ya = sbp.tile([128, 512], f16, tag="ya")
                        nc.scalar.activation(out=ya[:, :w], in_=y[:, :w],
                                             func=AF.Lrelu,
                                             bias=bias_sb[:, l:l + 1], scale=1.0,
                                             alpha=SLOPE)
                        sq = sbp.tile([128, 512], f16, tag="sq")
                        nc.vector.tensor_tensor(out=sq[:, :w], in0=ya[:, :w],
                                                in1=ya[:, :w], op=mult)
                        nsq = pnp.tile([1, 512], f32, tag="nsq")
                        nc.tensor.matmul(out=nsq[:, :w], lhsT=ones_c[:],
                                         rhs=sq[:, :w], start=True, stop=True)
                        rt = sbp.tile([1, 512], f16, tag="rt")
                        with nc.allow_low_precision(reason="fp16 norm"):
                            nc.scalar.activation(out=rt[:, :w], in_=nsq[:, :w],
                                                 func=AF.Sqrt)
                        bc = pbc.tile([128, 512], f32, tag="bc")
                        nc.tensor.matmul(out=bc[:, :w], lhsT=onesrow[:],
                                         rhs=rt[:, :w], start=True, stop=True)
                        bcm = sbp.tile([128, 512], f32, tag="bcm")
                        nc.vector.tensor_scalar(out=bcm[:, :w], in0=bc[:, :w],
                                                scalar1=EPS, scalar2=None,
                                                op0=maxop)
                        inv2 = sbp.tile([128, 512], f16, tag="inv2")
                        with nc.allow_low_precision(reason="fp16 norm recip"):
                            nc.vector.reciprocal(inv2[:, :w], bcm[:, :w])
                        nc.vector.tensor_tensor(out=ftout[:, b0:b0 + w],
                                                in0=ya[:, :w], in1=inv2[:, :w],
                                                op=mult)

                        # transpose + write out this block's rows
                        for h in range(b0, min(b0 + 512, 12544), 128):
                            nr = min(128, SHARD - h)
                            if nr <= 0:
                                break
                            tp = pwp.tile([128, 128], f16, tag="pw",
                                          name=f"tp{l}_{h}")
                            nc.tensor.transpose(out=tp[:],
                                                in_=ftout[:, h:h + 128],
                                                identity=ident[:])
                            cpo = sbp.tile([128, 128], f16, tag="cpo")
                            nc.vector.tensor_copy(cpo[:], tp[:])
                            nc.sync.dma_start(out=fshard[h:h + nr, :],
                                              in_=cpo[:nr, :])

                nc.gpsimd.collective_compute(
                    "AllGather", mybir.AluOpType.bypass,
                    replica_groups=[list(range(NCORE))],
                    ins=[fshard.opt()], outs=[ags[l].opt()],
                )

            # ---------- final gather + dot ----------
            acc_t = cp.tile([128, NFB], f32)
            for li, srcf in enumerate([feat16] + ags):
                ug = fp_.tile([128, NFB * 128], f16, tag="ug",
                              name=f"ug{li}")
                nc.gpsimd.dma_gather(
                    ug[:].rearrange("p (c d) -> p c d", d=128),
                    srcf[:],
                    uidx_sb[:],
                    NFB * 128, NFB * 128, 128,
                    single_packet=False, queue_num=(2 * li) % NQUEUES,
                )
                ig = fp_.tile([128, NFB * 128], f16, tag="ig",
                              name=f"ig{li}")
                for (rr, c0, cn) in fin_bounds:
                    nc.gpsimd.dma_gather(
                        ig[:, c0 * 128:(c0 + cn) * 128].rearrange(
                            "p (c d) -> p c d", d=128),
                        srcf[rr * RS:, :],
                        iidx_sb[:, c0 * 8:(c0 + cn) * 8],
                        cn * 128, cn * 128, 128,
                        single_packet=False, queue_num=(2 * li + 1 + rr) % NQUEUES,
                    )
                prod = fp_.tile([128, NFB * 128], f16, tag="prod",
                                name=f"prod{li}")
                nc.vector.tensor_tensor(out=prod[:], in0=ug[:], in1=ig[:],
                                        op=mult)
                sc = sbp.tile([128, NFB], f32, tag="sc")
                nc.vector.tensor_reduce(
                    out=sc[:],
                    in_=prod[:].rearrange("p (c d) -> p c d", d=128),
                    axis=mybir.AxisListType.X, op=add)
                if li == 0:
                    nc.vector.tensor_copy(acc_t[:], sc[:])
                else:
                    nc.vector.tensor_tensor(out=acc_t[:], in0=acc_t[:],
                                            in1=sc[:], op=add)
            nc.sync.dma_start(out=score[:], in_=acc_t[:])

    nc.compile()
    return nc


def _meta_key(meta):
    st, NFB, fin_bounds = meta
    return (st["windows"], st["chunks"], st["groups"], st["group_info"],
            tuple(st["o_off"].tolist()), NFB, fin_bounds)


def kernel(**inputs) -> np.ndarray:
    meta, in_maps, inv_perm = _pack_inputs(**inputs)
    key = _meta_key(meta)
    if key not in _cache:
        _cache[key] = _build(meta)
    nc = _cache[key]
    res = run_bass_kernel_spmd(nc, in_maps, list(range(NCORE)))
    out = np.empty(BATCH, dtype=np.float32)
    NFB = meta[1]
    for c in range(NCORE):
        sc = res.results[c]["score"]
        vals_j = sc[np.arange(NFB * 128) % 128, np.arange(NFB * 128) // 128]
        valid = inv_perm[c] >= 0
        out[inv_perm[c][valid]] = vals_j[valid]
    return out



# revision 13
# speedup vs baseline: 1.0405x; 1.0405x over previous
"""GCF message passing on 8 trn2 cores — windowed-SpMM redesign.

Per core (dest-shard of 12500 nodes), per layer:
  SpMM: edges sorted by dest, grouped into shared windows (<=128 dests);
        per (window, col-range) up to KMAX chunks of 128 edges. One-hot
        matrices O[slot, dest-d0] (vals folded, fp16) are precomputed on
        host and streamed from DRAM; gathered source rows (fp16, 256B)
        come from HBM dma_gather. PE accumulates Lx^T window = sum over
        chunks G^T @ O in PSUM; DVE casts to fp16 ftout.
  Dense: y^T = Wlin@(Lx+F)^T + Wint@(Lx*F)^T per 512-block, fused
        bias+leaky-relu on ACT, row norm via ones-matmul + reciprocal +
        broadcast-matmul, all fp16 in SBUF / f32 in PSUM.
  Share: PE-transpose shard -> fshard fp16, AllGather -> ags[l].
Final: per concat level gather u/i rows (fp16), multiply + reduce, acc.
"""

import os

import numpy as np

import concourse.bacc as bacc
import concourse.mybir as mybir
import concourse.tile as tile
from concourse.bass_utils import run_bass_kernel_spmd
from concourse.masks import make_identity

NUM_USERS = 30000
NUM_ITEMS = 70000
N = 100000
D = 128
NL = 3
BATCH = 16384
NCORE = 8
SHARD = N // NCORE
RS = 25000
NR = 4
WCAP = 128
KMAX = 3
GM = 48                # target chunks per gather group
VSCALE = 2550.0        # uint8 quantization scale for edge vals
BSH = BATCH // NCORE   # 2048
EPS = 1e-12
SLOPE = 0.01

f32 = mybir.dt.float32
f16 = mybir.dt.float16
i16 = mybir.dt.int16
u8 = mybir.dt.uint8

NQUEUES = int(os.environ.get("KQ", "4"))

_cache = {}


# ---------------------------------------------------------------- host side
def _build_structure(rows, cols):
    core = rows // SHARD
    dloc = rows - core * SHARD
    rng = cols // RS

    counts = np.zeros((NCORE, SHARD, NR), np.int32)
    np.add.at(counts, (core, dloc, rng), 1)

    windows = []
    cum = np.zeros((NCORE, NR), np.int64)
    d0 = 0
    for d in range(SHARD):
        c = counts[:, d, :]
        if (cum + c > 128 * KMAX).any() or d - d0 >= WCAP:
            windows.append((d0, d - d0))
            d0 = d
            cum = c.astype(np.int64).copy()
        else:
            cum += c
    windows.append((d0, SHARD - d0))
    nwin = len(windows)

    wsum = np.zeros((nwin, NR, NCORE), np.int64)
    for i, (a, w) in enumerate(windows):
        wsum[i] = counts[:, a:a + w, :].sum(axis=1).T
    kwr = np.ceil(wsum.max(axis=2) / 128).astype(np.int64)
    empty = kwr.sum(axis=1) == 0
    kwr[empty, 0] = 1

    chunks = []  # (win, r, j, is_first, is_last)
    for i in range(nwin):
        ks = kwr[i]
        tot = int(ks.sum())
        seen = 0
        for r in range(NR):
            for j in range(int(ks[r])):
                chunks.append((i, r, j, seen == 0, seen == tot - 1))
                seen += 1
    NCH = len(chunks)

    groups = []
    gstart, cnt = 0, 0
    for i in range(nwin):
        c = int(kwr[i].sum())
        if cnt + c > GM and cnt > 0:
            groups.append((gstart, i))
            gstart, cnt = i, 0
        cnt += c
    groups.append((gstart, nwin))

    ci_of = {}
    for ci, (i, r, j, _, _) in enumerate(chunks):
        ci_of[(i, r, j)] = ci
    chunk_group = np.zeros(NCH, np.int64)
    chunk_slot = np.zeros(NCH, np.int64)
    group_info = []
    for gi, (wa, wb) in enumerate(groups):
        col = 0
        rlist = []
        for r in range(NR):
            base, n_r = col, 0
            for i in range(wa, wb):
                for j in range(int(kwr[i, r])):
                    ci = ci_of[(i, r, j)]
                    chunk_group[ci] = gi
                    chunk_slot[ci] = col
                    col += 1
                    n_r += 1
            if n_r:
                rlist.append((r, base, n_r))
        group_info.append((col, tuple(rlist), (wa, wb)))

    o_off = np.zeros(NCH + 1, np.int64)
    for ci, (i, r, j, _, _) in enumerate(chunks):
        o_off[ci + 1] = o_off[ci] + windows[i][1]
    return dict(windows=tuple(windows), kwr=kwr, chunks=tuple(chunks),
                groups=tuple(groups), group_info=tuple(group_info),
                chunk_group=chunk_group, chunk_slot=chunk_slot,
                o_off=o_off, OW=int(o_off[-1]), NCH=NCH)


def _pack_edges(rows, cols, vals, st):
    core = rows // SHARD
    dloc = rows - core * SHARD
    rng = cols // RS
    windows, kwr = st["windows"], st["kwr"]
    nwin = len(windows)
    NCH = st["NCH"]

    win_of = np.zeros(SHARD, np.int64)
    d0s = np.zeros(nwin, np.int64)
    for i, (a, w) in enumerate(windows):
        win_of[a:a + w] = i
        d0s[i] = a
    win_e = win_of[dloc]

    first_chunk = np.zeros((nwin, NR), np.int64)
    acc = 0
    for i in range(nwin):
        for r in range(NR):
            first_chunk[i, r] = acc
            acc += int(kwr[i, r])

    key = (core * nwin + win_e) * NR + rng
    order = np.argsort(key, kind="stable")
    key_s = key[order]
    uniq, inv, cnt = np.unique(key_s, return_inverse=True, return_counts=True)
    starts = np.zeros(len(uniq), np.int64)
    np.cumsum(cnt[:-1], out=starts[1:])
    pos = np.arange(len(key_s)) - starts[inv]

    core_s = core[order]
    win_s = win_e[order]
    rng_s = rng[order]
    ci_e = first_chunk[win_s, rng_s] + pos // 128
    p_e = pos % 128

    gcols = np.array([g[0] for g in st["group_info"]], np.int64)
    gcol_base = np.zeros(len(gcols) + 1, np.int64)
    np.cumsum(gcols, out=gcol_base[1:])
    gslot = gcol_base[st["chunk_group"][ci_e]] + st["chunk_slot"][ci_e]

    eidx = np.zeros((NCORE, 16, NCH * 8), np.int16)
    col_local = (cols[order] - rng_s * RS).astype(np.int16)
    eidx[core_s, p_e % 16, gslot * 8 + p_e // 16] = col_local

    o_off = st["o_off"]
    O = np.zeros((NCORE, 128, st["OW"]), np.uint8)
    j_e = dloc[order] - d0s[win_s]
    q = np.clip(np.round(vals[order] * VSCALE), 0, 255).astype(np.uint8)
    O[core_s, p_e, o_off[ci_e] + j_e] = q
    return eidx, O


def _pack_final(userIdx, itemIdx):
    irow = itemIdx + NUM_USERS
    ir = irow // RS
    nfb_counts = np.zeros((NCORE, NR), np.int64)
    perms = []
    for c in range(NCORE):
        sl = slice(c * BSH, (c + 1) * BSH)
        o = np.argsort(ir[sl], kind="stable")
        perms.append(o)
        nfb_counts[c] = np.bincount(ir[sl][o], minlength=NR)
    bucket_chunks = np.ceil(nfb_counts.max(axis=0) / 128).astype(np.int64)
    fin_bounds = []
    c0 = 0
    for r in range(NR):
        n = int(bucket_chunks[r])
        if n == 0:
            continue
        fin_bounds.append((r, c0, n))
        c0 += n
    NFB = c0

    uidx_arr = np.zeros((NCORE, 16, NFB * 8), np.int16)
    iidx_arr = np.zeros((NCORE, 16, NFB * 8), np.int16)
    inv_perm = np.full((NCORE, NFB * 128), -1, np.int64)
    for c in range(NCORE):
        sl = slice(c * BSH, (c + 1) * BSH)
        o = perms[c]
        u_s = userIdx[sl][o]
        i_s = irow[sl][o]
        r_s = ir[sl][o]
        jpos = np.zeros(BSH, np.int64)
        for (r, b0, nchk) in fin_bounds:
            m = r_s == r
            jpos[m] = b0 * 128 + np.arange(int(m.sum()))
        uidx_arr[c, jpos % 16, (jpos // 128) * 8 + (jpos % 128) // 16] = \
            u_s.astype(np.int16)
        iidx_arr[c, jpos % 16, (jpos // 128) * 8 + (jpos % 128) // 16] = \
            (i_s - r_s * RS).astype(np.int16)
        inv_perm[c, jpos] = np.arange(c * BSH, (c + 1) * BSH)[o]
    return NFB, tuple(fin_bounds), uidx_arr, iidx_arr, inv_perm


def _pack_inputs(userIdx, itemIdx, rows, cols, vals, uEmbd, iEmbd,
                 Wlin, blin, Wint, bint):
    rows = np.asarray(rows, dtype=np.int64)
    cols = np.asarray(cols, dtype=np.int64)
    vals = np.asarray(vals, dtype=np.float32)
    userIdx = np.asarray(userIdx, dtype=np.int64)
    itemIdx = np.asarray(itemIdx, dtype=np.int64)

    feat0 = np.concatenate([np.asarray(uEmbd, np.float32),
                            np.asarray(iEmbd, np.float32)], axis=0)
    feat16 = np.ascontiguousarray(feat0.astype(np.float16))

    st = _build_structure(rows, cols)
    eidx, O = _pack_edges(rows, cols, vals, st)
    NFB, fin_bounds, uidx_arr, iidx_arr, inv_perm = _pack_final(
        userIdx, itemIdx)

    wlin_h = np.ascontiguousarray(
        np.asarray(Wlin, np.float32).transpose(1, 0, 2).reshape(D, NL * D)
    ).astype(np.float16)
    wint_h = np.ascontiguousarray(
        np.asarray(Wint, np.float32).transpose(1, 0, 2).reshape(D, NL * D)
    ).astype(np.float16)
    biasc = np.ascontiguousarray(
        (np.asarray(blin, np.float32) + np.asarray(bint, np.float32)).T)

    in_maps = []
    for c in range(NCORE):
        f0t = np.ascontiguousarray(
            feat16[c * SHARD:(c + 1) * SHARD].T)
        in_maps.append({
            "feat16": feat16,
            "f0t": f0t,
            "eidx": np.ascontiguousarray(np.tile(eidx[c], (8, 1))),
            "odram": np.ascontiguousarray(O[c]),
            "wlin": wlin_h,
            "wint": wint_h,
            "biasc": biasc,
            "uidx": np.ascontiguousarray(np.tile(uidx_arr[c], (8, 1))),
            "iidx": np.ascontiguousarray(np.tile(iidx_arr[c], (8, 1))),
        })

    meta = (st, NFB, fin_bounds)
    return meta, in_maps, inv_perm


# ---------------------------------------------------------------- kernel
def _build(meta):
    st, NFB, fin_bounds = meta
    windows = st["windows"]
    chunks = st["chunks"]
    group_info = st["group_info"]
    o_off = st["o_off"]
    kwr = st["kwr"]
    NCH = st["NCH"]
    OW = st["OW"]
    nwin = len(windows)

    gcols = [g[0] for g in group_info]
    maxgc = max(gcols)
    gcol_base = np.zeros(len(gcols) + 1, np.int64)
    np.cumsum(gcols, out=gcol_base[1:])

    # per-group O slice bounds (window-major chunk packing is contiguous)
    first_chunk = np.zeros((nwin, NR), np.int64)
    acc = 0
    for i in range(nwin):
        for r in range(NR):
            first_chunk[i, r] = acc
            acc += int(kwr[i, r])
    grp_obase = []
    grp_osize = []
    for (wa, wb) in st["groups"]:
        c_first = int(first_chunk[wa, 0])
        c_last = int(first_chunk[wb, 0]) if wb < nwin else NCH
        grp_obase.append(int(o_off[c_first]))
        grp_osize.append(int(o_off[c_last] - o_off[c_first]))
    maxgo = max(grp_osize)

    nc = bacc.Bacc(num_devices=NCORE, num_swdge_queues=max(NQUEUES, 1))

    feat16 = nc.dram_tensor("feat16", [N, D], f16, kind="ExternalInput")
    f0t = nc.dram_tensor("f0t", [D, SHARD], f16, kind="ExternalInput")
    eidx = nc.dram_tensor("eidx", [128, NCH * 8], i16, kind="ExternalInput")
    odram = nc.dram_tensor("odram", [128, OW], u8, kind="ExternalInput")
    wlin = nc.dram_tensor("wlin", [D, NL * D], f16, kind="ExternalInput")
    wint = nc.dram_tensor("wint", [D, NL * D], f16, kind="ExternalInput")
    biasc = nc.dram_tensor("biasc", [D, NL], f32, kind="ExternalInput")
    uidx = nc.dram_tensor("uidx", [128, NFB * 8], i16, kind="ExternalInput")
    iidx = nc.dram_tensor("iidx", [128, NFB * 8], i16, kind="ExternalInput")
    score = nc.dram_tensor("score", [128, NFB], f32, kind="ExternalOutput")

    add = mybir.AluOpType.add
    mult = mybir.AluOpType.mult
    maxop = mybir.AluOpType.max
    AF = mybir.ActivationFunctionType

    with tile.TileContext(nc) as tc:
        with (
            tc.tile_pool(name="const", bufs=1) as cp,
            tc.tile_pool(name="ft", bufs=1) as ftp,
            tc.tile_pool(name="g", bufs=4) as gp,
            tc.tile_pool(name="ob8", bufs=2) as op8,
            tc.tile_pool(name="ob", bufs=2) as op_,
            tc.tile_pool(name="sb", bufs=3) as sbp,
            tc.tile_pool(name="fin", bufs=2) as fp_,
            tc.tile_pool(name="pw", bufs=4, space="PSUM") as pwp,
            tc.tile_pool(name="py", bufs=2, space="PSUM") as pyp,
            tc.tile_pool(name="pn", bufs=1, space="PSUM") as pnp,
            tc.tile_pool(name="pbc", bufs=1, space="PSUM") as pbc,
            tc.tile_pool(name="dram", bufs=1, space="DRAM") as dp,
        ):
            # ---------- constants ----------
            ident = cp.tile([128, 128], f16)
            make_identity(nc, ident[:])
            ones_c = cp.tile([128, 1], f16)
            nc.vector.memset(ones_c[:], 1.0)
            onesrow = cp.tile([1, 128], f16)
            nc.vector.memset(onesrow[:], 1.0)

            wlin_sb = cp.tile([128, NL * 128], f16)
            nc.sync.dma_start(out=wlin_sb[:], in_=wlin[:])
            wint_sb = cp.tile([128, NL * 128], f16)
            nc.sync.dma_start(out=wint_sb[:], in_=wint[:])
            bias_sb = cp.tile([128, NL], f32)
            nc.sync.dma_start(out=bias_sb[:], in_=biasc[:])

            eidx_sb = cp.tile([128, NCH * 8], i16)
            nc.sync.dma_start(out=eidx_sb[:], in_=eidx[:])
            uidx_sb = cp.tile([128, NFB * 8], i16)
            nc.sync.dma_start(out=uidx_sb[:], in_=uidx[:])
            iidx_sb = cp.tile([128, NFB * 8], i16)
            nc.sync.dma_start(out=iidx_sb[:], in_=iidx[:])

            FTP = 12544  # SHARD padded to x128 for PE transposes
            fta = ftp.tile([128, FTP], f16, tag="fta")
            ftb = ftp.tile([128, FTP], f16, tag="ftb")
            nc.vector.memset(fta[:, SHARD:], 0.0)
            nc.vector.memset(ftb[:, SHARD:], 0.0)
            nc.sync.dma_start(out=fta[:, :SHARD], in_=f0t[:])

            fshards = [dp.tile([SHARD, D], f16, name=f"fsh{i}", tag=f"fsh{i}")
                       for i in range(2)]
            ags = [dp.tile([N, D], f16, name=f"ag{i}", tag=f"ag{i}",
                           addr_space="Shared") for i in range(NL)]

            acc_t = cp.tile([128, NFB], f32)

            def emit_final_level(li, srcf):
                ug = fp_.tile([128, NFB * 128], f16, tag="ug",
                              name=f"ug{li}")
                nc.gpsimd.dma_gather(
                    ug[:].rearrange("p (c d) -> p c d", d=128),
                    srcf[:],
                    uidx_sb[:],
                    NFB * 128, NFB * 128, 128,
                    single_packet=False, queue_num=(2 * li) % NQUEUES,
                )
                ig = fp_.tile([128, NFB * 128], f16, tag="ig",
                              name=f"ig{li}")
                for (rr, c0, cn) in fin_bounds:
                    nc.gpsimd.dma_gather(
                        ig[:, c0 * 128:(c0 + cn) * 128].rearrange(
                            "p (c d) -> p c d", d=128),
                        srcf[rr * RS:, :],
                        iidx_sb[:, c0 * 8:(c0 + cn) * 8],
                        cn * 128, cn * 128, 128,
                        single_packet=False,
                        queue_num=(2 * li + 1 + rr) % NQUEUES,
                    )
                prod = fp_.tile([128, NFB * 128], f16, tag="prod",
                                name=f"prod{li}")
                nc.vector.tensor_tensor(out=prod[:], in0=ug[:], in1=ig[:],
                                        op=mult)
                sc = sbp.tile([128, NFB], f32, tag="sc")
                nc.vector.tensor_reduce(
                    out=sc[:],
                    in_=prod[:].rearrange("p (c d) -> p c d", d=128),
                    axis=mybir.AxisListType.X, op=add)
                if li == 0:
                    nc.vector.tensor_copy(acc_t[:], sc[:])
                else:
                    nc.vector.tensor_tensor(out=acc_t[:], in0=acc_t[:],
                                            in1=sc[:], op=add)

            for l in range(NL):
                fshard = fshards[l % 2]
                ftin = fta if l % 2 == 0 else ftb
                ftout = ftb if l % 2 == 0 else fta
                src = feat16 if l == 0 else ags[l - 1]

                # ---------- SpMM ----------
                blocks_done = 0
                for gi, (ncols, rlist, (wa, wb)) in enumerate(group_info):
                    gt = gp.tile([128, maxgc * 128], f16, tag="gt",
                                 name=f"gt{l}_{gi}")
                    for (r, cbase, n_r) in rlist:
                        q = (r % NQUEUES) if NQUEUES > 1 else 0
                        nc.gpsimd.dma_gather(
                            gt[:, cbase * 128:(cbase + n_r) * 128].rearrange(
                                "p (c d) -> p c d", d=128),
                            src[r * RS:, :],
                            eidx_sb[:, (gcol_base[gi] + cbase) * 8:
                                    (gcol_base[gi] + cbase + n_r) * 8],
                            n_r * 128, n_r * 128, 128,
                            single_packet=False, queue_num=q,
                        )
                    ob8 = op8.tile([128, maxgo], u8, tag="ob8",
                                   name=f"ob8{l}_{gi}")
                    nc.scalar.dma_start(
                        out=ob8[:, :grp_osize[gi]],
                        in_=odram[:, grp_obase[gi]:
                                  grp_obase[gi] + grp_osize[gi]])
                    ob = op_.tile([128, maxgo], f16, tag="ob",
                                  name=f"ob{l}_{gi}")
                    nc.vector.tensor_copy(ob[:, :grp_osize[gi]],
                                          ob8[:, :grp_osize[gi]])

                    for i in range(wa, wb):
                        d0, w = windows[i]
                        ps = pwp.tile([128, w], f32, tag="pw",
                                      name=f"pw{l}_{i}")
                        for r in range(NR):
                            for j in range(int(kwr[i, r])):
                                ci = int(first_chunk[i, r]) + j
                                _, _, _, isf, isl = chunks[ci]
                                sc = int(st["chunk_slot"][ci])
                                oloc = int(o_off[ci]) - grp_obase[gi]
                                nc.tensor.matmul(
                                    out=ps[:],
                                    lhsT=gt[:, sc * 128:(sc + 1) * 128],
                                    rhs=ob[:, oloc:oloc + w],
                                    start=isf, stop=isl,
                                )
                        if i % 2 == 0:
                            nc.vector.tensor_scalar(
                                out=ftout[:, d0:d0 + w], in0=ps[:],
                                scalar1=1.0 / VSCALE, scalar2=None, op0=mult)
                        else:
                            nc.scalar.activation(
                                out=ftout[:, d0:d0 + w], in_=ps[:],
                                func=AF.Copy, scale=1.0 / VSCALE)

                    # ---------- dense blocks now fully covered ----------
                    if gi + 1 < len(group_info):
                        nw = group_info[gi + 1][2][0]
                        dest_end = windows[nw][0]
                    else:
                        dest_end = SHARD
                    while (blocks_done + 512 <= dest_end
                           or (gi + 1 == len(group_info)
                               and blocks_done < SHARD)):
                        b0 = blocks_done
                        blocks_done += 512
                        w = min(512, SHARD - b0)
                        lxs = ftout[:, b0:b0 + w]
                        fin_ = ftin[:, b0:b0 + w]
                        pre1 = sbp.tile([128, 512], f16, tag="pre1")
                        nc.vector.tensor_tensor(out=pre1[:, :w], in0=lxs,
                                                in1=fin_, op=add)
                        pre2 = sbp.tile([128, 512], f16, tag="pre2")
                        nc.vector.tensor_tensor(out=pre2[:, :w], in0=lxs,
                                                in1=fin_, op=mult)
                        y = pyp.tile([128, 512], f32, tag="y")
                        nc.tensor.matmul(out=y[:, :w],
                                         lhsT=wlin_sb[:, l * 128:(l + 1) * 128],
                                         rhs=pre1[:, :w], start=True, stop=False)
                        nc.tensor.matmul(out=y[:, :w],
                                         lhsT=wint_sb[:, l * 128:(l + 1) * 128],
                                         rhs=pre2[:, :w], start=False, stop=True)
                        ya = sbp.tile([128, 512], f16, tag="ya")
                        nc.scalar.activation(out=ya[:, :w], in_=y[:, :w],
                                             func=AF.Lrelu,
                                             bias=bias_sb[:, l:l + 1], scale=1.0,
                                             alpha=SLOPE)
                        sq = sbp.tile([128, 512], f16, tag="sq")
                        nc.vector.tensor_tensor(out=sq[:, :w], in0=ya[:, :w],
                                                in1=ya[:, :w], op=mult)
                        nsq = pnp.tile([1, 512], f32, tag="nsq")
                        nc.tensor.matmul(out=nsq[:, :w], lhsT=ones_c[:],
                                         rhs=sq[:, :w], start=True, stop=True)
                        rt = sbp.tile([1, 512], f16, tag="rt")
                        with nc.allow_low_precision(reason="fp16 norm"):
                            nc.scalar.activation(out=rt[:, :w], in_=nsq[:, :w],
                                                 func=AF.Abs_reciprocal_sqrt)
                        bc = pbc.tile([128, 512], f32, tag="bc")
                        nc.tensor.matmul(out=bc[:, :w], lhsT=onesrow[:],
                                         rhs=rt[:, :w], start=True, stop=True)
                        nc.vector.tensor_tensor(out=ftout[:, b0:b0 + w],
                                                in0=ya[:, :w], in1=bc[:, :w],
                                                op=mult)

                        # transpose + write out this block's rows
                        for h in range(b0, min(b0 + 512, 12544), 128):
                            nr = min(128, SHARD - h)
                            if nr <= 0:
                                break
                            tp = pwp.tile([128, 128], f16, tag="pw",
                                          name=f"tp{l}_{h}")
                            nc.tensor.transpose(out=tp[:],
                                                in_=ftout[:, h:h + 128],
                                                identity=ident[:])
                            cpo = sbp.tile([128, 128], f16, tag="cpo")
                            nc.vector.tensor_copy(cpo[:], tp[:])
                            nc.sync.dma_start(out=fshard[h:h + nr, :],
                                              in_=cpo[:nr, :])

                emit_final_level(l, feat16 if l == 0 else ags[l - 1])
                nc.gpsimd.collective_compute(
                    "AllGather", mybir.AluOpType.bypass,
                    replica_groups=[list(range(NCORE))],
                    ins=[fshard.opt()], outs=[ags[l].opt()],
                )

            emit_final_level(NL, ags[NL - 1])
            nc.sync.dma_start(out=score[:], in_=acc_t[:])

    nc.compile()
    return nc


def _meta_key(meta):
    st, NFB, fin_bounds = meta
    return (st["windows"], st["chunks"], st["groups"], st["group_info"],
            tuple(st["o_off"].tolist()), NFB, fin_bounds)


def kernel(**inputs) -> np.ndarray:
    meta, in_maps, inv_perm = _pack_inputs(**inputs)
    key = _meta_key(meta)
    if key not in _cache:
        _cache[key] = _build(meta)
    nc = _cache[key]
    res = run_bass_kernel_spmd(nc, in_maps, list(range(NCORE)))
    out = np.empty(BATCH, dtype=np.float32)
    NFB = meta[1]
    for c in range(NCORE):
        sc = res.results[c]["score"]
        vals_j = sc[np.arange(NFB * 128) % 128, np.arange(NFB * 128) // 128]
        valid = inv_perm[c] >= 0
        out[inv_perm[c][valid]] = vals_j[valid]
    return out

